# revision 12
# baseline (speedup 1.0000x reference)
"""Trainium2 Bass kernel for nn_Loss_Synonymy.

reference:
    diff = S1 - S2                       # [B, 256]
    d    = sqrt(sum(diff^2, axis=-1))    # [B]
    t    = tanh(d)
    err  = where(score >= 0.8, relu(1 - t), relu(1 + t))
    out  = sum(err) / B

Since tanh(d) in [0, 1) for d >= 0, relu(1 -+ tanh(d)) = 1 -+ tanh(d), so
err = 1 + sgn * tanh(d) and sum(err) = B + sum(sgn * tanh(d)).  The
kernel only accumulates sgn * tanh(d); the host adds B and divides.

Data-parallel over 8 NeuronCores, 32768 rows each.  Partition p owns
rows [p*256, (p+1)*256) of the shard, so the score vector is ONE
contiguous [128, 256] load and per-row sums land as [128, 256] aligned
with it.  s1/s2 are stacked host-side into x[2, BL, D] so each tile is
a single dma_start.

The tile stream is a casting SWDGE DMA (f32 HBM -> bf16 SBUF): HBM
traffic is unchanged but every on-chip pass runs on half the bytes and
tensor_sub gets the DVE 2x bf16 tier (tensor_reduce is 1x-capped
regardless).  bf16 before the subtract is safe: diff ~ N(0, sqrt(2)),
same scale as the inputs, and tanh(d~16) is saturated.

Per big tile (J=16 row-chunks per partition, KD reduced on DVE):
    SWDGE: X[128, 2*J*256] bf16 <- x[:, p*256+off .. +J, :] (cast)
    DVE  : diff = a - b  (bf16 2x, in place into the b half)
    ACT  : Square rows [0, KD) in place; rows [KD, J) squared with
           accum_out straight into their sumsq column (per-row)
    DVE  : sumsq[:, off:off+KD] = reduce_add(sq.view(128, KD, 256))
The DVE reduce of tile t is emitted after sub of tile t+1 so the
in-order DVE never waits on ACT.  4 J=4 taper tiles shrink the drain.

Epilogue: d = sumsq * min(rsqrt(sumsq), 1e6)  (Abs_reciprocal_sqrt
avoids the Sqrt table set; the clamp makes sumsq==0 give d=0 exactly
like the reference), th = Tanh(d), then (score >= 0.8 ? -1 : +1) * th
accumulated per partition -> [128, 1].
Host: out = (B + sum(partials)) / B.
"""

import numpy as np

import concourse.bass as bass
import concourse.tile as tile
from concourse import bacc, mybir
from concourse.bass_utils import run_bass_kernel_spmd

F32 = mybir.dt.float32
BF16 = mybir.dt.bfloat16
AF = mybir.ActivationFunctionType
ALU = mybir.AluOpType

B = 262144
D = 256
NCORES = 8
BL = B // NCORES          # 32768 rows per core
RPP = BL // 128           # 256 rows per partition
THRESH = 0.8

# (J, count, KD): per-partition row-chunks per tile; sum(J*count) == RPP.
# KD rows are row-sum-reduced on DVE (tensor_reduce, 1x-capped), J-KD on
# ACT (per-row Square+accum ~0.85us each incl READ_ACCUMULATOR).
TILING = [(16, 15, 13), (4, 4, 3)]
BIG_J = TILING[0][0]
BUFS_X = 6
BUFS_XS = 4

_NC_CACHE = {}


def _build_nc():
    nc = bacc.Bacc(
        "TRN2", target_bir_lowering=False, debug=False, num_devices=NCORES
    )

    x = nc.dram_tensor("x", [2, BL, D], F32, kind="ExternalInput").ap()
    score = nc.dram_tensor("score", [BL], F32, kind="ExternalInput").ap()
    partial = nc.dram_tensor("partial", [128, 1], F32, kind="ExternalOutput").ap()

    # [128, 2, 256, 256]: partition p / source s / row-in-block c / feature d
    x_r = x.rearrange("s (p c) d -> p s c d", p=128, c=RPP)
    score_r = score.rearrange("(p c) -> p c", p=128, c=RPP)

    with tile.TileContext(nc) as tc:
        with (
            tc.tile_pool(name="xin", bufs=BUFS_X) as p_x,
            tc.tile_pool(name="xsmall", bufs=BUFS_XS) as p_xs,
            tc.tile_pool(name="persist", bufs=1) as p_per,
        ):
            sumsq = p_per.tile([128, RPP], F32, tag="sumsq")
            score_sb = p_per.tile([128, RPP], F32, tag="score_sb")
            part_sb = p_per.tile([128, 1], F32, tag="part_sb")
            sgn2 = p_per.tile([128, RPP], F32, tag="sgn2")

            # Discarded elementwise output of the ACT accum rows. Raw sbuf
            # tensor (not a pool tile) so Tile's tracking ignores it.
            scr_act = nc.alloc_sbuf_tensor("scr_act", [128, D], BF16).ap()

            pending = None  # (X_bf16, off, KD) awaiting its DVE reduce

            def emit_reduce(p):
                Xb, off, KD = p
                nc.vector.tensor_reduce(
                    sumsq[:, off : off + KD],
                    Xb[:, 0 : KD * D].rearrange("p (j d) -> p j d", d=D),
                    axis=mybir.AxisListType.X,
                    op=ALU.add,
                )

            off = 0
            first = True
            for J, count, KD in TILING:
                FREE = J * D
                big = J == BIG_J
                for _ in range(count):
                    X = (p_x if big else p_xs).tile(
                        [128, 2 * FREE], BF16, tag=f"x{J}"
                    )
                    # casting DMA: f32 in HBM -> bf16 in SBUF (SWDGE-only)
                    nc.gpsimd.dma_start(
                        X[:].rearrange("p (s j d) -> p s j d", s=2, d=D),
                        x_r[:, :, off : off + J, :],
                    )
                    if first:
                        # Score: one contiguous [128, 256] load; HWDGE ring
                        # so it doesn't sit in front of the tile stream.
                        nc.sync.dma_start(score_sb[:], score_r)
                        nc.vector.tensor_scalar(
                            sgn2[:], score_sb[:], THRESH, -2.0,
                            ALU.is_ge, ALU.mult,
                        )
                        first = False
                    a = X[:, 0:FREE]
                    b = X[:, FREE : 2 * FREE]
                    nc.vector.tensor_sub(b, a, b)
                    nc.scalar.activation(
                        b[:, 0 : KD * D], b[:, 0 : KD * D], AF.Square
                    )
                    for i in range(KD, J):
                        nc.scalar.activation(
                            scr_act,
                            b[:, i * D : (i + 1) * D],
                            AF.Square,
                            accum_out=sumsq[:, off + i : off + i + 1],
                        )
                    if pending is not None:
                        emit_reduce(pending)
                    pending = (b, off, KD)
                    off += J
            emit_reduce(pending)

            # Epilogue: part = sum_p sgn * tanh(d), d = sumsq * rsqrt(sumsq).
            rs = p_per.tile([128, RPP], F32, tag="rs")
            nc.scalar.activation(rs[:], sumsq[:], AF.Abs_reciprocal_sqrt)
            # min(rs, 1e6) clamps rsqrt(0)=inf so sumsq==0 -> dist=0 -> tanh=0,
            # exactly matching the reference for degenerate rows.
            dist = p_per.tile([128, RPP], F32, tag="dist")
            nc.vector.scalar_tensor_tensor(
                dist[:], rs[:], 1e6, sumsq[:], ALU.min, ALU.mult
            )
            th = p_per.tile([128, RPP], F32, tag="th")
            nc.scalar.activation(th[:], dist[:], AF.Tanh)
            # (sgn2 + 1) * th -> +-tanh, accumulated per partition
            err = p_per.tile([128, RPP], F32, tag="err")
            nc.vector.scalar_tensor_tensor(
                err[:], sgn2[:], 1.0, th[:], ALU.add, ALU.mult,
                accum_out=part_sb[:],
            )

            nc.sync.dma_start(partial, part_sb[:])

    nc.compile()
    return nc


def _get_nc():
    if "nc" not in _NC_CACHE:
        _NC_CACHE["nc"] = _build_nc()
    return _NC_CACHE["nc"]


def make_in_maps(S1_out, S2_out, synonymy_score):
    in_maps = []
    for c in range(NCORES):
        lo, hi = c * BL, (c + 1) * BL
        x = np.empty((2, BL, D), dtype=np.float32)
        x[0] = S1_out[lo:hi]
        x[1] = S2_out[lo:hi]
        in_maps.append(
            {
                "x": x,
                "score": np.ascontiguousarray(
                    synonymy_score[lo:hi], dtype=np.float32
                ),
            }
        )
    return in_maps


def combine(results):
    total = np.float64(B)
    for r in results:
        total += r["partial"].astype(np.float64).sum()
    return np.asarray(total / B, dtype=np.float32)


def run(S1_out, S2_out, synonymy_score, trace=False, **trace_kwargs):
    nc = _get_nc()
    in_maps = make_in_maps(S1_out, S2_out, synonymy_score)
    res = run_bass_kernel_spmd(
        nc, in_maps, list(range(NCORES)), trace=trace, **trace_kwargs
    )
    return combine(res.results), res


def kernel(S1_out, S2_out, synonymy_score):
    out, _ = run(S1_out, S2_out, synonymy_score)
    return out
